# revision 10
# baseline (speedup 1.0000x reference)
"""Trainium2 Bass kernel for nn_CNNQNetwork (dueling CNN Q-network).

Sharding: pure data parallel — batch 4096 split as 512 samples on each of the
8 NeuronCores; all weights replicated.

Per-core layout: activations live in SBUF as [channel(partition), batch,
spatial] (b-major, s innermost) so squares/reduces are packed-bf16 DVE ops.

Per block (conv + GroupNorm(1 group) + relu):
  - Sum(z) over (C,S) per sample is computed BEFORE the conv runs, from the
    conv *input*, via column-sum weights:  sum_cs conv(u) =
    sum_t wsum_t . window_sum_t(u).  window sums are cheap packed bf16 DVE
    reduces of the parent activation; the per-tap dot is a tiny K-dim matmul.
  - The mean subtraction is folded into the conv's own PSUM accumulation
    group as a K=1 ones-matmul with a broadcast rhs, so the PSUM holds
    (z - mu) directly and Var = sum((z-mu)^2)/CS with no -mu^2 correction.
  - squares -> bf16 [c,b,s] (packed) -> DVE reduce -> ones-matmul -> Sqrt on
    ACT -> one reciprocal per block -> per-sample scale row r.
  - feat = u * (gamma_c * r_b) via one broadcast tensor_tensor per block
    (split across DVE / GpSimd to balance engines).
The dueling-head algebra (v + a - mean(a), biases) is folded into the second
linear layer's weights on the host; head weights are prefetched at kernel
start so their 7.6MB DMA overlaps the conv phase.

Math notes used for exactness (verified against the reference):
  - relu(GroupNorm) with gamma>0, beta=0, conv bias=0 allows deferring the
    per-sample 1/std into the *feature* tensor only; intermediate blocks are
    scale invariant because GroupNorm(conv(r*u)) == GroupNorm(conv(u)).
  - per-channel gamma of h1/v1 is folded into the consuming conv weights.
"""

import numpy as np
import ml_dtypes

BF16 = ml_dtypes.bfloat16
B_TOTAL = 4096
NCORES = 8
BC = B_TOTAL // NCORES  # 512 samples per core
D = 128
EPS = 1e-5

# blocks: (name, src, kind, Hi, Wi, Ho, Wo)   kind 'h' = (1,2) kernel, 'v' = (2,1)
BLOCKS = [
    ("h1", "x2", "h", 4, 4, 4, 3),
    ("v1", "x3", "v", 4, 4, 3, 4),
    ("hh", "h1", "h", 4, 3, 4, 2),
    ("hv", "h1", "v", 4, 3, 3, 3),
    ("vh", "v1", "h", 3, 4, 3, 3),
    ("vv", "v1", "v", 3, 4, 2, 4),
]
S_OF = {n: ho * wo for (n, _, _, _, _, ho, wo) in BLOCKS}
NK = sum(S_OF.values())  # 58 K-slices of 128 for the head matmul

# squares must run on ACT (DVE cannot read PSUM twice in one op); balance
# engines by moving some blocks' relu to DVE as tensor_scalar_max
RELU_ON_DVE = {"h1", "hh", "vh"}
# which engine does the feat = u*G broadcast multiply
FEAT_ON_GPSIMD = {"h1", "v1"}

_cache = {}


def _build(loop_n=None):
    """Build the Bass program once. Returns nc."""
    import concourse.bass as bass
    import concourse.tile as tile
    import concourse.mybir as mybir
    from concourse import bacc
    from concourse.masks import make_identity
    from contextlib import ExitStack, nullcontext

    dt = mybir.dt
    Alu = mybir.AluOpType
    Act = mybir.ActivationFunctionType

    nc = bacc.Bacc(
        "TRN2",
        target_bir_lowering=False,
        debug=False,
        enable_asserts=False,
        num_devices=NCORES,
    )

    # ---- DRAM I/O ----
    x2_d = nc.dram_tensor("x2", [32, BC, 16], dt.bfloat16, kind="ExternalInput")
    x3_d = nc.dram_tensor("x3", [32, BC, 16], dt.bfloat16, kind="ExternalInput")
    cw1_d = nc.dram_tensor("cw1", [32, 256], dt.bfloat16, kind="ExternalInput")
    cw_d = nc.dram_tensor("cw", [128, 8 * 128], dt.bfloat16, kind="ExternalInput")
    ws1_d = nc.dram_tensor("ws1", [32, 2], dt.bfloat16, kind="ExternalInput")
    wsc_d = nc.dram_tensor("wsc", [128, 8], dt.bfloat16, kind="ExternalInput")
    hw_d = nc.dram_tensor("hw", [4, 128, NK * 128], dt.bfloat16, kind="ExternalInput")
    fw_d = nc.dram_tensor("fw", [128, 16], dt.bfloat16, kind="ExternalInput")
    hb_d = nc.dram_tensor("hb", [128, 4], dt.float32, kind="ExternalInput")
    b2_d = nc.dram_tensor("b2", [4, 1], dt.float32, kind="ExternalInput")
    gam_d = nc.dram_tensor("gam", [1, 6 * 128], dt.bfloat16, kind="ExternalInput")
    out_d = nc.dram_tensor("out", [BC, 4], dt.float32, kind="ExternalOutput")

    with tile.TileContext(nc) as tc, ExitStack() as ctx:
        singles = ctx.enter_context(tc.tile_pool(name="singles", bufs=1))
        rows = ctx.enter_context(tc.tile_pool(name="rows", bufs=2))
        sqp = ctx.enter_context(tc.tile_pool(name="sqp", bufs=4))
        up = ctx.enter_context(tc.tile_pool(name="up", bufs=1))
        ubp = ctx.enter_context(tc.tile_pool(name="ubp", bufs=1))
        zsp = ctx.enter_context(tc.tile_pool(name="zsp", bufs=2))
        gsp = ctx.enter_context(tc.tile_pool(name="gsp", bufs=2))

        # persistent SBUF tensors
        fw_sb = singles.tile([128, 16], dt.bfloat16, tag="fw", name="fw")
        hb_sb = singles.tile([128, 4], dt.float32, tag="hb", name="hb")
        b2_sb = singles.tile([4, 1], dt.float32, tag="b2", name="b2")
        gam_sb = singles.tile([1, 6 * 128], dt.bfloat16, tag="gam", name="gam")
        ident = singles.tile([128, 128], dt.float32, tag="ident", name="ident")
        onesr = singles.tile([1, 128], dt.bfloat16, tag="onesr", name="onesr")
        onesc = singles.tile([128, 1], dt.bfloat16, tag="onesc", name="onesc")
        eps1 = singles.tile([1, 1], dt.float32, tag="eps1", name="eps1")
        nc.vector.memset(eps1[:], EPS)
        nc.vector.memset(onesr[:], 1.0)
        nc.vector.memset(onesc[:], 1.0)

        x2_sb = singles.tile([32, BC, 16], dt.bfloat16, tag="x2", name="x2")
        x3_sb = singles.tile([32, BC, 16], dt.bfloat16, tag="x3", name="x3")
        cw1_sb = singles.tile([32, 256], dt.bfloat16, tag="cw1", name="cw1")
        cw_sb = singles.tile([128, 8 * 128], dt.bfloat16, tag="cw", name="cw")
        ws1_sb = singles.tile([32, 2], dt.bfloat16, tag="ws1", name="ws1")
        wsc_sb = singles.tile([128, 8], dt.bfloat16, tag="wsc", name="wsc")
        nc.sync.dma_start(x2_sb[:], x2_d[:])
        nc.sync.dma_start(x3_sb[:], x3_d[:])
        nc.sync.dma_start(cw1_sb[:], cw1_d[:])
        nc.sync.dma_start(cw_sb[:], cw_d[:])
        nc.sync.dma_start(ws1_sb[:], ws1_d[:])
        nc.sync.dma_start(wsc_sb[:], wsc_d[:])
        nc.sync.dma_start(fw_sb[:], fw_d[:])
        nc.sync.dma_start(hb_sb[:], hb_d[:])
        nc.sync.dma_start(b2_sb[:], b2_d[:])
        nc.sync.dma_start(gam_sb[:], gam_d[:])
        make_identity(nc, ident[:])

        # head weights: prefetch all 4 m-tiles now; DMA overlaps conv phase
        hws = []
        for mt in range(4):
            h = singles.tile([128, NK * 128], dt.bfloat16, tag=f"hw{mt}", name=f"hw{mt}")
            nc.sync.dma_start(h[:], hw_d[mt])
            hws.append(h)

        # u (pre-scale) and feat (scaled) activations, [c, b, s]
        u_keep = {
            "h1": singles.tile([128, BC, 12], dt.bfloat16, tag="u_h1", name="u_h1"),
            "v1": singles.tile([128, BC, 12], dt.bfloat16, tag="u_v1", name="u_v1"),
        }
        feat = {}
        for name, _, _, _, _, ho, wo in BLOCKS:
            feat[name] = singles.tile(
                [128, BC, ho * wo], dt.bfloat16, tag=f"f_{name}", name=f"f_{name}"
            )

        with (tc.For_i(0, loop_n, 1) if loop_n else nullcontext()):
            with (
                tc.tile_pool(name="zp", bufs=5, space="PSUM") as zp,
                tc.tile_pool(name="sp", bufs=2, space="PSUM") as sp,
                tc.tile_pool(name="gp", bufs=1, space="PSUM") as gp,
            ):
                for bi, (name, src, kind, Hi, Wi, Ho, Wo) in enumerate(BLOCKS):
                    S = Ho * Wo
                    CS = 128 * S
                    first = src in ("x2", "x3")

                    if first:
                        sview = (x2_sb if src == "x2" else x3_sb)[:].rearrange(
                            "c b (i j) -> c b i j", i=Hi
                        )
                    else:
                        sview = u_keep[src][:].rearrange("c b (i j) -> c b i j", i=Hi)

                    # ---- Sum(z) over (C,S) per sample, from the conv input ----
                    psSz = sp.tile([1, BC], dt.float32, tag="ps", name="psSz")
                    if first:
                        # taps already stacked into the 32 input channels
                        if kind == "h":
                            win = sview[:, :, :, 0:Wo]
                        else:
                            win = sview[:, :, 0:Ho, :]
                        Ut = up.tile([32, BC], dt.float32, tag="U", name="U")
                        nc.vector.tensor_reduce(
                            Ut[:], win, axis=mybir.AxisListType.XY, op=Alu.add
                        )
                        Utb = ubp.tile([32, BC], dt.bfloat16, tag="Ub", name="Ub")
                        nc.vector.tensor_copy(Utb[:], Ut[:])
                        nc.tensor.matmul(
                            psSz[:], ws1_sb[:, bi : bi + 1], Utb[:],
                            start=True, stop=True,
                        )
                    else:
                        for t in range(2):
                            if kind == "h":
                                win = sview[:, :, :, t : t + Wo]
                            else:
                                win = sview[:, :, t : t + Ho, :]
                            Ut = up.tile([128, BC], dt.float32, tag="U", name="U")
                            nc.vector.tensor_reduce(
                                Ut[:], win, axis=mybir.AxisListType.XY, op=Alu.add
                            )
                            Utb = ubp.tile([128, BC], dt.bfloat16, tag="Ub", name="Ub")
                            nc.vector.tensor_copy(Utb[:], Ut[:])
                            col = (bi - 2) * 2 + t
                            nc.tensor.matmul(
                                psSz[:], wsc_sb[:, col : col + 1], Utb[:],
                                start=(t == 0), stop=(t == 1),
                            )
                    # negz = -mean = -Sum(z)/CS, bf16 row for the K=1 matmul rhs
                    negz = rows.tile([1, BC], dt.bfloat16, tag="negz", name="negz")
                    with nc.allow_low_precision("bf16 mean row"):
                        nc.vector.tensor_scalar_mul(negz[:], psSz[:], -1.0 / CS)

                    # leaf blocks: relu writes into feat and the gamma*r
                    # multiply is done in place (saves a full leaf-u tensor)
                    u_dst = u_keep[name] if name in u_keep else feat[name]
                    zs2 = zsp.tile([128, BC], dt.bfloat16, tag="zs2", name="zs2")

                    relu_dve = name in RELU_ON_DVE
                    for g in range(16):
                        b0 = g * 32
                        zc = zp.tile([128, 32, S], dt.float32, tag="z", name="z")
                        zc4 = zc[:].rearrange("c b (i j) -> c b i j", i=Ho)
                        if first:
                            lhsT = cw1_sb[:, bi * 128 : bi * 128 + 128]
                            if kind == "h":
                                rhs = sview[:, b0 : b0 + 32, :, 0:Wo]
                            else:
                                rhs = sview[:, b0 : b0 + 32, 0:Ho, :]
                            nc.tensor.matmul(zc4, lhsT, rhs, start=True, stop=False)
                        else:
                            t0 = (bi - 2) * 2
                            for t in range(2):
                                lhsT = cw_sb[:, (t0 + t) * 128 : (t0 + t + 1) * 128]
                                if kind == "h":
                                    rhs = sview[:, b0 : b0 + 32, :, t : t + Wo]
                                else:
                                    rhs = sview[:, b0 : b0 + 32, t : t + Ho, :]
                                nc.tensor.matmul(
                                    zc4, lhsT, rhs, start=(t == 0), stop=False
                                )
                        # mean subtraction folded into the accumulation group
                        nc.tensor.matmul(
                            zc[:],
                            onesr[:],
                            negz[:, b0 : b0 + 32][:, :, None].to_broadcast((1, 32, S)),
                            start=False,
                            stop=True,
                        )
                        # squares of (z-mu), bf16 packed [c,b,s] — ACT only
                        # (DVE cannot read two PSUM operands in one op)
                        sq = sqp.tile([128, 32, S], dt.bfloat16, tag="sq", name="sq")
                        nc.scalar.square(sq[:], zc[:])
                        with nc.allow_low_precision("bf16 var partial sums"):
                            nc.vector.tensor_reduce(
                                zs2[:, b0 : b0 + 32], sq[:],
                                axis=mybir.AxisListType.X, op=Alu.add,
                            )
                        # u = relu(z - mu)
                        if relu_dve:
                            with nc.allow_low_precision("bf16 relu copy"):
                                nc.vector.tensor_scalar_max(
                                    u_dst[:, b0 : b0 + 32, :], zc[:], 0.0
                                )
                        else:
                            nc.scalar.activation(
                                u_dst[:, b0 : b0 + 32, :], zc[:], func=Act.Relu
                            )

                    # ---- per-sample scale r = 1/sqrt(Var+eps), G = gamma x r ----
                    psSq = sp.tile([1, BC], dt.float32, tag="ps", name="psSq")
                    nc.tensor.matmul(psSq[:], onesc[:], zs2[:], start=True, stop=True)
                    sdrow = rows.tile([1, BC], dt.float32, tag="sdrow", name="sdrow")
                    nc.scalar.activation(
                        sdrow[:], psSq[:], func=Act.Sqrt, bias=eps1[:], scale=1.0 / CS
                    )
                    rrow = rows.tile([1, BC], dt.float32, tag="rrow", name="rrow")
                    nc.vector.reciprocal(rrow[:], sdrow[:])
                    rrowb = rows.tile([1, BC], dt.bfloat16, tag="rrowb", name="rrowb")
                    nc.vector.tensor_copy(rrowb[:], rrow[:])
                    psG = gp.tile([128, BC], dt.float32, tag="psG", name="psG")
                    nc.tensor.matmul(
                        psG[:], gam_sb[:, bi * 128 : (bi + 1) * 128], rrowb[:],
                        start=True, stop=True,
                    )
                    gsb = gsp.tile([128, BC], dt.bfloat16, tag="gsb", name="gsb")
                    nc.scalar.copy(gsb[:], psG[:])
                    gbc = gsb[:, :, None].to_broadcast((128, BC, S))
                    if name in FEAT_ON_GPSIMD:
                        nc.gpsimd.tensor_tensor(feat[name][:], u_dst[:], gbc, op=Alu.mult)
                    else:
                        nc.vector.tensor_tensor(feat[name][:], u_dst[:], gbc, op=Alu.mult)

            # ---- heads ----
            with (
                tc.tile_pool(name="hidp", bufs=1) as hidp,
                tc.tile_pool(name="hp", bufs=2, space="PSUM") as hp,
                tc.tile_pool(name="fp", bufs=1, space="PSUM") as fp,
                tc.tile_pool(name="tp", bufs=2, space="PSUM") as tp,
            ):
                hids = []
                for mt in range(4):
                    psH = hp.tile([128, BC], dt.float32, tag="psH", name="psH")
                    k = 0
                    for name, _, _, _, _, ho, wo in BLOCKS:
                        for s in range(ho * wo):
                            nc.tensor.matmul(
                                psH[:],
                                hws[mt][:, k * 128 : (k + 1) * 128],
                                feat[name][:, :, s],
                                start=(k == 0),
                                stop=(k == NK - 1),
                            )
                            k += 1
                    hid = hidp.tile([128, BC], dt.bfloat16, tag=f"hid{mt}", name=f"hid{mt}")
                    nc.scalar.activation(
                        hid[:], psH[:], func=Act.Relu, bias=hb_sb[:, mt : mt + 1], scale=1.0
                    )
                    hids.append(hid)
                psF = fp.tile([4, BC], dt.float32, tag="psF", name="psF")
                for mt in range(4):
                    nc.tensor.matmul(
                        psF[:],
                        fw_sb[:, mt * 4 : (mt + 1) * 4],
                        hids[mt][:],
                        start=(mt == 0),
                        stop=(mt == 3),
                    )
                finf = rows.tile([4, BC], dt.float32, tag="finf", name="finf")
                nc.scalar.activation(
                    finf[:], psF[:], func=Act.Identity, bias=b2_sb[:, 0:1], scale=1.0
                )
                osb = rows.tile([128, 4, 4], dt.float32, tag="osb", name="osb")
                for qq in range(4):
                    psT = tp.tile([128, 4], dt.float32, tag="psT", name="psT")
                    nc.tensor.transpose(
                        psT[:], finf[:, qq * 128 : (qq + 1) * 128], ident[0:4, 0:4]
                    )
                    nc.scalar.copy(osb[:, qq, :], psT[:])
                nc.sync.dma_start(out_d[:].rearrange("(q p) j -> p q j", p=128), osb[:])

    nc.compile()
    return nc


def _prep_weights(inp):
    """Host-side weight preprocessing shared by all cores."""
    f32 = np.float32
    for k in ("b_h1", "b_v1", "b_hh", "b_hv", "b_vh", "b_vv"):
        assert np.allclose(inp[k], 0.0), f"conv bias {k} must be zero"
    for k in ("gb_h1", "gb_v1", "gb_hh", "gb_hv", "gb_vh", "gb_vv"):
        assert np.allclose(inp[k], 0.0), f"groupnorm beta {k} must be zero"
    gammas = {n: np.asarray(inp[f"gw_{n}"], f32) for n in S_OF}
    for n, g in gammas.items():
        assert np.all(g > 0), f"gamma {n} must be positive"

    # first-level conv lhsT (taps stacked into K=32)
    w_h1 = np.asarray(inp["w_h1"], f32)
    w_v1 = np.asarray(inp["w_v1"], f32)
    cw1 = np.zeros((32, 256), f32)
    cw1[0:16, 0:128] = w_h1[:, :, 0, 0].T
    cw1[16:32, 0:128] = w_h1[:, :, 0, 1].T
    cw1[0:16, 128:256] = w_v1[:, :, 0, 0].T
    cw1[16:32, 128:256] = w_v1[:, :, 1, 0].T

    # second-level conv lhsT with parent's gamma folded in
    cw = np.zeros((128, 8 * 128), f32)
    second = [
        ("hh", "w_hh", "h1", "h"),
        ("hv", "w_hv", "h1", "v"),
        ("vh", "w_vh", "v1", "h"),
        ("vv", "w_vv", "v1", "v"),
    ]
    for idx, (name, wk, parent, kind) in enumerate(second):
        w = np.asarray(inp[wk], f32)
        g = gammas[parent]
        for t in range(2):
            tap = w[:, :, 0, t] if kind == "h" else w[:, :, t, 0]
            cw[:, (2 * idx + t) * 128 : (2 * idx + t + 1) * 128] = (tap * g[None, :]).T

    # column-sum weights for the Sum(z) trick.  Sums are over the bf16
    # lhsT actually used on device, so the statistic matches the conv.
    cw1b = cw1.astype(BF16).astype(f32)
    cwb = cw.astype(BF16).astype(f32)
    ws1 = np.zeros((32, 2), f32)
    ws1[:, 0] = cw1b[:, 0:128].sum(axis=1)
    ws1[:, 1] = cw1b[:, 128:256].sum(axis=1)
    wsc = np.zeros((128, 8), f32)
    for col in range(8):
        wsc[:, col] = cwb[:, col * 128 : (col + 1) * 128].sum(axis=1)

    # head weights: W1c = [vw1; aw1] (512, 7424), re-tiled per (mtile, block, s)
    W1c = np.concatenate(
        [np.asarray(inp["vw1"], f32), np.asarray(inp["aw1"], f32)], axis=0
    )
    cols = []
    off = 0
    for name, _, _, _, _, ho, wo in BLOCKS:
        S = ho * wo
        Wb = W1c[:, off : off + 128 * S].reshape(512, 128, S)
        off += 128 * S
        for s in range(S):
            cols.append(Wb[:, :, s])
    K = np.stack(cols, 0)  # (58, 512, 128c)
    hw = np.empty((4, 128, NK * 128), f32)
    for mt in range(4):
        hw[mt] = K[:, mt * 128 : (mt + 1) * 128, :].transpose(2, 0, 1).reshape(128, -1)

    # final layer with dueling algebra folded in
    vw2 = np.asarray(inp["vw2"], f32)  # (1, 256)
    aw2 = np.asarray(inp["aw2"], f32)  # (4, 256)
    W2c = np.zeros((4, 512), f32)
    W2c[:, 0:256] = vw2[0][None, :]
    W2c[:, 256:512] = aw2 - aw2.mean(axis=0, keepdims=True)
    W2cT = W2c.T  # (512, 4)
    fw = np.zeros((128, 16), f32)
    for kt in range(4):
        fw[:, kt * 4 : (kt + 1) * 4] = W2cT[kt * 128 : (kt + 1) * 128, :]
    b2 = (
        np.asarray(inp["vb2"], f32)[0]
        + np.asarray(inp["ab2"], f32)
        - np.asarray(inp["ab2"], f32).mean()
    ).reshape(4, 1)

    hb = np.concatenate(
        [np.asarray(inp["vb1"], f32), np.asarray(inp["ab1"], f32)]
    ).reshape(4, 128).T.copy()  # [128, 4], column mt

    gam = np.zeros((1, 6 * 128), f32)
    for bi, (name, _, _, _, _, _, _) in enumerate(BLOCKS):
        gam[0, bi * 128 : (bi + 1) * 128] = gammas[name]

    return {
        "cw1": cw1.astype(BF16),
        "cw": cw.astype(BF16),
        "ws1": ws1.astype(BF16),
        "wsc": wsc.astype(BF16),
        "hw": hw.astype(BF16),
        "fw": fw.astype(BF16),
        "hb": hb.astype(np.float32),
        "b2": b2.astype(np.float32),
        "gam": gam.astype(BF16),
    }


def _prep_x(xs):
    """Per-core input prep: tap-stacked, [c, b, s] bf16 arrays."""
    f32 = np.float32
    n = xs.shape[0]
    x2 = np.zeros((n, 32, 4, 4), f32)
    x2[:, 0:16] = xs
    x2[:, 16:32, :, 0:3] = xs[:, :, :, 1:4]
    x3 = np.zeros((n, 32, 4, 4), f32)
    x3[:, 0:16] = xs
    x3[:, 16:32, 0:3, :] = xs[:, :, 1:4, :]
    x2 = x2.transpose(1, 0, 2, 3).reshape(32, n, 16)
    x3 = x3.transpose(1, 0, 2, 3).reshape(32, n, 16)
    return x2.astype(BF16), x3.astype(BF16)


def _get_nc():
    if "nc" not in _cache:
        _cache["nc"] = _build()
    return _cache["nc"]


def kernel(**inputs) -> np.ndarray:
    from concourse.bass_utils import run_bass_kernel_spmd

    nc = _get_nc()
    x = np.asarray(inputs["x"], np.float32)
    w = _prep_weights(inputs)

    in_maps = []
    for c in range(NCORES):
        xs = x[c * BC : (c + 1) * BC]
        x2, x3 = _prep_x(xs)
        m = dict(w)
        m["x2"] = x2
        m["x3"] = x3
        in_maps.append(m)

    res = run_bass_kernel_spmd(nc, in_maps, core_ids=list(range(NCORES)))
    out = np.concatenate([r["out"] for r in res.results], axis=0)
    return out.astype(np.float32)


# revision 13
# speedup vs baseline: 1.4748x; 1.4748x over previous
"""Trainium2 Bass kernel for nn_CNNQNetwork (dueling CNN Q-network).

Sharding: pure data parallel — batch 4096 split as 512 samples on each of the
8 NeuronCores; all weights replicated.

Per-core layout: activations live in SBUF as [channel(partition), spatial,
batch] so conv-window and head matmul rhs are contiguous along batch.

Per block (conv + GroupNorm(1 group) + relu):
  - Sum(z) over (C,S) per sample is computed BEFORE the conv runs, from the
    conv *input*, via column-sum weights:  sum_cs conv(u) =
    sum_t wsum_t . window_sum_t(u).  Window sums of the parent activation are
    short chains of packed bf16 DVE adds over contiguous [c, b] s-slices
    (precomputed on the host for the first level); the per-tap dot is a tiny
    K-dim matmul.
  - The mean subtraction is folded into the conv's own PSUM accumulation
    group as a K=1 ones-matmul with a broadcast rhs, so the PSUM holds
    (z - mu) directly and Var = sum((z-mu)^2)/CS with no -mu^2 correction.
  - squares on ACT write transposed bf16 [c,b,s] tiles so the DVE variance
    reduce runs in packed 2-byte mode; relu is split ACT/DVE to balance.
  - Sum over channels uses an all-ones [128,128] lhsT so Var lands on all
    128 partitions, making sqrt / reciprocal_approx_fast / gamma-scale all
    full-width single ops (no 1-partition row math).
  - feat = u * (gamma_c * r_b) via one broadcast tensor_tensor per block
    (GpSimd for h1/v1, DVE in-place for the leaf blocks).
The dueling-head algebra (v + a - mean(a), biases) is folded into the second
linear layer's weights on the host; head weight m-tiles stream through a
double-buffered pool so their DMA hides under the previous tile's matmuls.

Math notes used for exactness (verified against the reference):
  - relu(GroupNorm) with gamma>0, beta=0, conv bias=0 allows deferring the
    per-sample 1/std into the *feature* tensor only; intermediate blocks are
    scale invariant because GroupNorm(conv(r*u)) == GroupNorm(conv(u)).
  - per-channel gamma of h1/v1 is folded into the consuming conv weights.
"""

import numpy as np
import ml_dtypes

BF16 = ml_dtypes.bfloat16
B_TOTAL = 4096
NCORES = 8
BC = B_TOTAL // NCORES  # 512 samples per core
D = 128
EPS = 1e-5

# blocks: (name, src, kind, Hi, Wi, Ho, Wo)   kind 'h' = (1,2) kernel, 'v' = (2,1)
BLOCKS = [
    ("h1", "x2", "h", 4, 4, 4, 3),
    ("v1", "x3", "v", 4, 4, 3, 4),
    ("hh", "h1", "h", 4, 3, 4, 2),
    ("hv", "h1", "v", 4, 3, 3, 3),
    ("vh", "v1", "h", 3, 4, 3, 3),
    ("vv", "v1", "v", 3, 4, 2, 4),
]
S_OF = {n: ho * wo for (n, _, _, _, _, ho, wo) in BLOCKS}
NK = sum(S_OF.values())  # 58 K-slices of 128 for the head matmul

# squares must run on ACT (DVE cannot read PSUM twice in one op); balance
# engines by moving some blocks' relu to DVE as tensor_scalar_max
RELU_ON_DVE = {"h1", "hh", "vh"}
# which engine does the feat = u*G broadcast multiply
FEAT_ON_GPSIMD = {"h1", "v1"}

_cache = {}


def _build(loop_n=None):
    """Build the Bass program once. Returns nc."""
    import concourse.bass as bass
    import concourse.tile as tile
    import concourse.mybir as mybir
    from concourse import bacc
    from concourse.masks import make_identity
    from contextlib import ExitStack, nullcontext

    dt = mybir.dt
    Alu = mybir.AluOpType
    Act = mybir.ActivationFunctionType

    nc = bacc.Bacc(
        "TRN2",
        target_bir_lowering=False,
        debug=False,
        enable_asserts=False,
        num_devices=NCORES,
    )

    # ---- DRAM I/O ----
    x2_d = nc.dram_tensor("x2", [32, 16, BC], dt.bfloat16, kind="ExternalInput")
    x3_d = nc.dram_tensor("x3", [32, 16, BC], dt.bfloat16, kind="ExternalInput")
    u2_d = nc.dram_tensor("u2", [32, BC], dt.bfloat16, kind="ExternalInput")
    u3_d = nc.dram_tensor("u3", [32, BC], dt.bfloat16, kind="ExternalInput")
    cw1_d = nc.dram_tensor("cw1", [32, 256], dt.bfloat16, kind="ExternalInput")
    cw_d = nc.dram_tensor("cw", [128, 8 * 128], dt.bfloat16, kind="ExternalInput")
    ws1_d = nc.dram_tensor("ws1", [32, 2], dt.bfloat16, kind="ExternalInput")
    wsc_d = nc.dram_tensor("wsc", [128, 8], dt.bfloat16, kind="ExternalInput")
    hw_d = nc.dram_tensor("hw", [4, 128, NK * 128], dt.bfloat16, kind="ExternalInput")
    fw_d = nc.dram_tensor("fw", [128, 16], dt.bfloat16, kind="ExternalInput")
    hb_d = nc.dram_tensor("hb", [128, 4], dt.float32, kind="ExternalInput")
    b2_d = nc.dram_tensor("b2", [4, 1], dt.float32, kind="ExternalInput")
    gamc_d = nc.dram_tensor("gamc", [128, 6], dt.float32, kind="ExternalInput")
    out_d = nc.dram_tensor("out", [BC, 4], dt.float32, kind="ExternalOutput")

    with tile.TileContext(nc) as tc, ExitStack() as ctx:
        singles = ctx.enter_context(tc.tile_pool(name="singles", bufs=1))
        rows = ctx.enter_context(tc.tile_pool(name="rows", bufs=2))
        sqp = ctx.enter_context(tc.tile_pool(name="sqp", bufs=4))
        colp = ctx.enter_context(tc.tile_pool(name="colp", bufs=2))
        statp = ctx.enter_context(tc.tile_pool(name="statp", bufs=2))
        zsp = ctx.enter_context(tc.tile_pool(name="zsp", bufs=2))
        gsp = ctx.enter_context(tc.tile_pool(name="gsp", bufs=2))
        hwp = ctx.enter_context(tc.tile_pool(name="hwp", bufs=2))

        # persistent SBUF tensors
        fw_sb = singles.tile([128, 16], dt.bfloat16, tag="fw", name="fw")
        hb_sb = singles.tile([128, 4], dt.float32, tag="hb", name="hb")
        b2_sb = singles.tile([4, 1], dt.float32, tag="b2", name="b2")
        gamc_sb = singles.tile([128, 6], dt.float32, tag="gamc", name="gamc")
        ident = singles.tile([128, 128], dt.float32, tag="ident", name="ident")
        onesr = singles.tile([1, 128], dt.bfloat16, tag="onesr", name="onesr")
        ones2 = singles.tile([128, 128], dt.bfloat16, tag="ones2", name="ones2")
        eps1 = singles.tile([1, 1], dt.float32, tag="eps1", name="eps1")
        epsc = singles.tile([128, 1], dt.float32, tag="epsc", name="epsc")
        nc.vector.memset(eps1[:], EPS)
        nc.vector.memset(epsc[:], EPS)
        nc.vector.memset(onesr[:], 1.0)
        nc.vector.memset(ones2[:], 1.0)

        x2_sb = singles.tile([32, 16, BC], dt.bfloat16, tag="x2", name="x2")
        x3_sb = singles.tile([32, 16, BC], dt.bfloat16, tag="x3", name="x3")
        u2_sb = singles.tile([32, BC], dt.bfloat16, tag="u2", name="u2")
        u3_sb = singles.tile([32, BC], dt.bfloat16, tag="u3", name="u3")
        cw1_sb = singles.tile([32, 256], dt.bfloat16, tag="cw1", name="cw1")
        cw_sb = singles.tile([128, 8 * 128], dt.bfloat16, tag="cw", name="cw")
        ws1_sb = singles.tile([32, 2], dt.bfloat16, tag="ws1", name="ws1")
        wsc_sb = singles.tile([128, 8], dt.bfloat16, tag="wsc", name="wsc")
        nc.sync.dma_start(x2_sb[:], x2_d[:])
        nc.sync.dma_start(x3_sb[:], x3_d[:])
        nc.sync.dma_start(u2_sb[:], u2_d[:])
        nc.sync.dma_start(u3_sb[:], u3_d[:])
        nc.sync.dma_start(cw1_sb[:], cw1_d[:])
        nc.sync.dma_start(cw_sb[:], cw_d[:])
        nc.sync.dma_start(ws1_sb[:], ws1_d[:])
        nc.sync.dma_start(wsc_sb[:], wsc_d[:])
        nc.sync.dma_start(fw_sb[:], fw_d[:])
        nc.sync.dma_start(hb_sb[:], hb_d[:])
        nc.sync.dma_start(b2_sb[:], b2_d[:])
        nc.sync.dma_start(gamc_sb[:], gamc_d[:])
        make_identity(nc, ident[:])

        # head weights: double-buffered stream; first two DMAs overlap convs
        hws = {}
        for mt in range(2):
            h = hwp.tile([128, NK * 128], dt.bfloat16, tag="hw", name=f"hw{mt}")
            nc.sync.dma_start(h[:], hw_d[mt])
            hws[mt] = h

        # u (pre-scale) and feat (scaled) activations, [c, s, b]
        u_keep = {
            "h1": singles.tile([128, 12, BC], dt.bfloat16, tag="u_h1", name="u_h1"),
            "v1": singles.tile([128, 12, BC], dt.bfloat16, tag="u_v1", name="u_v1"),
        }
        feat = {}
        for name, _, _, _, _, ho, wo in BLOCKS:
            feat[name] = singles.tile(
                [128, ho * wo, BC], dt.bfloat16, tag=f"f_{name}", name=f"f_{name}"
            )
        # per-(child, tap) window sums of the parent activation
        Usum = {}
        for child, tap in [(c, t) for c in ("hh", "hv", "vh", "vv") for t in (0, 1)]:
            Usum[(child, tap)] = singles.tile(
                [128, BC], dt.bfloat16, tag=f"U_{child}{tap}", name=f"U_{child}{tap}"
            )

        def parent_colsums(pname):
            """Build Usum[(child, tap)] rows for both children of a parent."""
            pu = u_keep[pname]
            _, _, _, _, _, Ho, Wo = next(b for b in BLOCKS if b[0] == pname)
            sl = lambda s: pu[:, s, :]

            def add2(dst, a, b_):
                nc.vector.tensor_tensor(dst, a, b_, op=Alu.add)

            with nc.allow_low_precision("bf16 window sums"):
                # column sums over rows, per column j
                cols = []
                for j in range(Wo):
                    c = colp.tile([128, BC], dt.bfloat16, tag=f"cs{j}", name=f"cs_{pname}{j}")
                    add2(c[:], sl(j), sl(Wo + j))
                    for i in range(2, Ho):
                        add2(c[:], c[:], sl(i * Wo + j))
                    cols.append(c)
                full = colp.tile([128, BC], dt.bfloat16, tag="full", name=f"full_{pname}")
                add2(full[:], cols[0][:], cols[1][:])
                for j in range(2, Wo):
                    add2(full[:], full[:], cols[j][:])
                # h-child (1x2 kernel): window = cols t..t+Wo-2
                hchild = {"h1": "hh", "v1": "vh"}[pname]
                if Wo == 3:  # windows are col pairs
                    add2(Usum[(hchild, 0)][:], cols[0][:], cols[1][:])
                    add2(Usum[(hchild, 1)][:], cols[1][:], cols[2][:])
                else:  # Wo == 4: full minus edge column
                    nc.vector.tensor_tensor(
                        Usum[(hchild, 0)][:], full[:], cols[3][:], op=Alu.subtract
                    )
                    nc.vector.tensor_tensor(
                        Usum[(hchild, 1)][:], full[:], cols[0][:], op=Alu.subtract
                    )
                # v-child (2x1 kernel): window = rows t..t+Ho-2
                vchild = {"h1": "hv", "v1": "vv"}[pname]
                if Ho == 3:  # row pairs
                    for t in (0, 1):
                        r = Usum[(vchild, t)]
                        add2(r[:], sl(t * Wo), sl(t * Wo + 1))
                        for j in range(2, Wo):
                            add2(r[:], r[:], sl(t * Wo + j))
                        add2(r[:], r[:], sl((t + 1) * Wo))
                        for j in range(1, Wo):
                            add2(r[:], r[:], sl((t + 1) * Wo + j))
                else:  # Ho == 4: full minus edge row
                    for t in (0, 1):
                        edge = 3 * Wo if t == 0 else 0
                        e = colp.tile([128, BC], dt.bfloat16, tag="edge", name=f"e_{pname}{t}")
                        add2(e[:], sl(edge), sl(edge + 1))
                        for j in range(2, Wo):
                            add2(e[:], e[:], sl(edge + j))
                        nc.vector.tensor_tensor(
                            Usum[(vchild, t)][:], full[:], e[:], op=Alu.subtract
                        )

        with (tc.For_i(0, loop_n, 1) if loop_n else nullcontext()):
            with (
                tc.tile_pool(name="zp", bufs=5, space="PSUM") as zp,
                tc.tile_pool(name="sp", bufs=2, space="PSUM") as sp,
                tc.tile_pool(name="gp", bufs=1, space="PSUM") as gp,
            ):
                for bi, (name, src, kind, Hi, Wi, Ho, Wo) in enumerate(BLOCKS):
                    S = Ho * Wo
                    CS = 128 * S
                    first = src in ("x2", "x3")

                    if first:
                        sview = (x2_sb if src == "x2" else x3_sb)[:].rearrange(
                            "c (i j) b -> c i j b", i=Hi
                        )
                    else:
                        sview = u_keep[src][:].rearrange("c (i j) b -> c i j b", i=Hi)

                    # ---- Sum(z) over (C,S) per sample, from the conv input ----
                    if name == "v1":
                        parent_colsums("h1")
                    elif name == "hh":
                        parent_colsums("v1")
                    psSz = sp.tile([1, BC], dt.float32, tag="ps", name="psSz")
                    if first:
                        usrc = u2_sb if src == "x2" else u3_sb
                        nc.tensor.matmul(
                            psSz[:], ws1_sb[:, bi : bi + 1], usrc[:],
                            start=True, stop=True,
                        )
                    else:
                        for t in range(2):
                            col = (bi - 2) * 2 + t
                            nc.tensor.matmul(
                                psSz[:], wsc_sb[:, col : col + 1], Usum[(name, t)][:],
                                start=(t == 0), stop=(t == 1),
                            )
                    # negz = -mean = -Sum(z)/CS, bf16 row for the K=1 matmul rhs
                    negz = rows.tile([1, BC], dt.bfloat16, tag="negz", name="negz")
                    with nc.allow_low_precision("bf16 mean row"):
                        nc.vector.tensor_scalar_mul(negz[:], psSz[:], -1.0 / CS)

                    # leaf blocks: relu writes into feat and the gamma*r
                    # multiply is done in place (saves a full leaf-u tensor)
                    u_dst = u_keep[name] if name in u_keep else feat[name]
                    zs2 = zsp.tile([128, BC], dt.bfloat16, tag="zs2", name="zs2")

                    relu_dve = name in RELU_ON_DVE
                    for g in range(16):
                        b0 = g * 32
                        zc = zp.tile([128, S, 32], dt.float32, tag="z", name="z")
                        zc4 = zc[:].rearrange("c (i j) b -> c i j b", i=Ho)
                        if first:
                            lhsT = cw1_sb[:, bi * 128 : bi * 128 + 128]
                            if kind == "h":
                                rhs = sview[:, :, 0:Wo, b0 : b0 + 32]
                            else:
                                rhs = sview[:, 0:Ho, :, b0 : b0 + 32]
                            nc.tensor.matmul(zc4, lhsT, rhs, start=True, stop=False)
                        else:
                            t0 = (bi - 2) * 2
                            for t in range(2):
                                lhsT = cw_sb[:, (t0 + t) * 128 : (t0 + t + 1) * 128]
                                if kind == "h":
                                    rhs = sview[:, :, t : t + Wo, b0 : b0 + 32]
                                else:
                                    rhs = sview[:, t : t + Ho, :, b0 : b0 + 32]
                                nc.tensor.matmul(
                                    zc4, lhsT, rhs, start=(t == 0), stop=False
                                )
                        # mean subtraction folded into the accumulation group
                        nc.tensor.matmul(
                            zc[:],
                            onesr[:],
                            negz[:, None, b0 : b0 + 32].to_broadcast((1, S, 32)),
                            start=False,
                            stop=True,
                        )
                        # squares of (z-mu): ACT writes transposed so the DVE
                        # variance reduce reads packed bf16 [c,b,s]
                        sq = sqp.tile([128, 32, S], dt.bfloat16, tag="sq", name="sq")
                        nc.scalar.square(sq[:].rearrange("c b s -> c s b"), zc[:])
                        with nc.allow_low_precision("bf16 var partial sums"):
                            nc.vector.tensor_reduce(
                                zs2[:, b0 : b0 + 32], sq[:],
                                axis=mybir.AxisListType.X, op=Alu.add,
                            )
                        # u = relu(z - mu)
                        if relu_dve:
                            with nc.allow_low_precision("bf16 relu copy"):
                                nc.vector.tensor_scalar_max(
                                    u_dst[:, :, b0 : b0 + 32], zc[:], 0.0
                                )
                        else:
                            nc.scalar.activation(
                                u_dst[:, :, b0 : b0 + 32], zc[:], func=Act.Relu
                            )

                    # ---- per-sample scale r = 1/sqrt(Var+eps), G = gamma x r ----
                    # all-ones lhsT puts Var on all 128 partitions
                    psSq = gp.tile([128, BC], dt.float32, tag="psSq", name="psSq")
                    nc.tensor.matmul(psSq[:], ones2[:], zs2[:], start=True, stop=True)
                    sdf = statp.tile([128, BC], dt.float32, tag="sdf", name="sdf")
                    nc.scalar.activation(
                        sdf[:], psSq[:], func=Act.Sqrt, bias=epsc[:], scale=1.0 / CS
                    )
                    rf = statp.tile([128, BC], dt.float32, tag="rf", name="rf")
                    nc.vector.reciprocal_approx_fast(rf[:], sdf[:])
                    gsb = gsp.tile([128, BC], dt.bfloat16, tag="gsb", name="gsb")
                    with nc.allow_low_precision("bf16 scale row"):
                        nc.vector.tensor_scalar_mul(
                            gsb[:], rf[:], gamc_sb[:, bi : bi + 1]
                        )
                    gbc = gsb[:, None, :].to_broadcast((128, S, BC))
                    if name in FEAT_ON_GPSIMD:
                        nc.gpsimd.tensor_tensor(feat[name][:], u_dst[:], gbc, op=Alu.mult)
                    else:
                        nc.vector.tensor_tensor(feat[name][:], u_dst[:], gbc, op=Alu.mult)

            # ---- heads ----
            with (
                tc.tile_pool(name="hidp", bufs=1) as hidp,
                tc.tile_pool(name="hp", bufs=2, space="PSUM") as hp,
                tc.tile_pool(name="fp", bufs=1, space="PSUM") as fp,
                tc.tile_pool(name="tp", bufs=2, space="PSUM") as tp,
            ):
                hids = []
                for mt in range(4):
                    if mt not in hws:
                        h = hwp.tile([128, NK * 128], dt.bfloat16, tag="hw", name=f"hw{mt}")
                        nc.sync.dma_start(h[:], hw_d[mt])
                        hws[mt] = h
                    psH = hp.tile([128, BC], dt.float32, tag="psH", name="psH")
                    k = 0
                    for name, _, _, _, _, ho, wo in BLOCKS:
                        for s in range(ho * wo):
                            nc.tensor.matmul(
                                psH[:],
                                hws[mt][:, k * 128 : (k + 1) * 128],
                                feat[name][:, s, :],
                                start=(k == 0),
                                stop=(k == NK - 1),
                            )
                            k += 1
                    hid = hidp.tile([128, BC], dt.bfloat16, tag=f"hid{mt}", name=f"hid{mt}")
                    nc.scalar.activation(
                        hid[:], psH[:], func=Act.Relu, bias=hb_sb[:, mt : mt + 1], scale=1.0
                    )
                    hids.append(hid)
                psF = fp.tile([4, BC], dt.float32, tag="psF", name="psF")
                for mt in range(4):
                    nc.tensor.matmul(
                        psF[:],
                        fw_sb[:, mt * 4 : (mt + 1) * 4],
                        hids[mt][:],
                        start=(mt == 0),
                        stop=(mt == 3),
                    )
                finf = rows.tile([4, BC], dt.float32, tag="finf", name="finf")
                nc.scalar.activation(
                    finf[:], psF[:], func=Act.Identity, bias=b2_sb[:, 0:1], scale=1.0
                )
                osb = rows.tile([128, 4, 4], dt.float32, tag="osb", name="osb")
                for qq in range(4):
                    psT = tp.tile([128, 4], dt.float32, tag="psT", name="psT")
                    nc.tensor.transpose(
                        psT[:], finf[:, qq * 128 : (qq + 1) * 128], ident[0:4, 0:4]
                    )
                    nc.scalar.copy(osb[:, qq, :], psT[:])
                nc.sync.dma_start(out_d[:].rearrange("(q p) j -> p q j", p=128), osb[:])

    nc.compile()
    return nc


def _prep_weights(inp):
    """Host-side weight preprocessing shared by all cores."""
    f32 = np.float32
    for k in ("b_h1", "b_v1", "b_hh", "b_hv", "b_vh", "b_vv"):
        assert np.allclose(inp[k], 0.0), f"conv bias {k} must be zero"
    for k in ("gb_h1", "gb_v1", "gb_hh", "gb_hv", "gb_vh", "gb_vv"):
        assert np.allclose(inp[k], 0.0), f"groupnorm beta {k} must be zero"
    gammas = {n: np.asarray(inp[f"gw_{n}"], f32) for n in S_OF}
    for n, g in gammas.items():
        assert np.all(g > 0), f"gamma {n} must be positive"

    # first-level conv lhsT (taps stacked into K=32)
    w_h1 = np.asarray(inp["w_h1"], f32)
    w_v1 = np.asarray(inp["w_v1"], f32)
    cw1 = np.zeros((32, 256), f32)
    cw1[0:16, 0:128] = w_h1[:, :, 0, 0].T
    cw1[16:32, 0:128] = w_h1[:, :, 0, 1].T
    cw1[0:16, 128:256] = w_v1[:, :, 0, 0].T
    cw1[16:32, 128:256] = w_v1[:, :, 1, 0].T

    # second-level conv lhsT with parent's gamma folded in
    cw = np.zeros((128, 8 * 128), f32)
    second = [
        ("hh", "w_hh", "h1", "h"),
        ("hv", "w_hv", "h1", "v"),
        ("vh", "w_vh", "v1", "h"),
        ("vv", "w_vv", "v1", "v"),
    ]
    for idx, (name, wk, parent, kind) in enumerate(second):
        w = np.asarray(inp[wk], f32)
        g = gammas[parent]
        for t in range(2):
            tap = w[:, :, 0, t] if kind == "h" else w[:, :, t, 0]
            cw[:, (2 * idx + t) * 128 : (2 * idx + t + 1) * 128] = (tap * g[None, :]).T

    # column-sum weights for the Sum(z) trick.  Sums are over the bf16
    # lhsT actually used on device, so the statistic matches the conv.
    cw1b = cw1.astype(BF16).astype(f32)
    cwb = cw.astype(BF16).astype(f32)
    ws1 = np.zeros((32, 2), f32)
    ws1[:, 0] = cw1b[:, 0:128].sum(axis=1)
    ws1[:, 1] = cw1b[:, 128:256].sum(axis=1)
    wsc = np.zeros((128, 8), f32)
    for col in range(8):
        wsc[:, col] = cwb[:, col * 128 : (col + 1) * 128].sum(axis=1)

    # head weights: W1c = [vw1; aw1] (512, 7424), re-tiled per (mtile, block, s)
    W1c = np.concatenate(
        [np.asarray(inp["vw1"], f32), np.asarray(inp["aw1"], f32)], axis=0
    )
    cols = []
    off = 0
    for name, _, _, _, _, ho, wo in BLOCKS:
        S = ho * wo
        Wb = W1c[:, off : off + 128 * S].reshape(512, 128, S)
        off += 128 * S
        for s in range(S):
            cols.append(Wb[:, :, s])
    K = np.stack(cols, 0)  # (58, 512, 128c)
    hw = np.empty((4, 128, NK * 128), f32)
    for mt in range(4):
        hw[mt] = K[:, mt * 128 : (mt + 1) * 128, :].transpose(2, 0, 1).reshape(128, -1)

    # final layer with dueling algebra folded in
    vw2 = np.asarray(inp["vw2"], f32)  # (1, 256)
    aw2 = np.asarray(inp["aw2"], f32)  # (4, 256)
    W2c = np.zeros((4, 512), f32)
    W2c[:, 0:256] = vw2[0][None, :]
    W2c[:, 256:512] = aw2 - aw2.mean(axis=0, keepdims=True)
    W2cT = W2c.T  # (512, 4)
    fw = np.zeros((128, 16), f32)
    for kt in range(4):
        fw[:, kt * 4 : (kt + 1) * 4] = W2cT[kt * 128 : (kt + 1) * 128, :]
    b2 = (
        np.asarray(inp["vb2"], f32)[0]
        + np.asarray(inp["ab2"], f32)
        - np.asarray(inp["ab2"], f32).mean()
    ).reshape(4, 1)

    hb = np.concatenate(
        [np.asarray(inp["vb1"], f32), np.asarray(inp["ab1"], f32)]
    ).reshape(4, 128).T.copy()  # [128, 4], column mt

    gamc = np.zeros((128, 6), f32)
    for bi, (name, _, _, _, _, _, _) in enumerate(BLOCKS):
        gamc[:, bi] = gammas[name]

    return {
        "cw1": cw1.astype(BF16),
        "cw": cw.astype(BF16),
        "ws1": ws1.astype(BF16),
        "wsc": wsc.astype(BF16),
        "hw": hw.astype(BF16),
        "fw": fw.astype(BF16),
        "hb": hb.astype(np.float32),
        "b2": b2.astype(np.float32),
        "gamc": gamc.astype(np.float32),
    }


def _prep_x(xs):
    """Per-core input prep: tap-stacked [c, s, b] bf16 arrays plus the
    first-level window sums (h1: cols 0..2, v1: rows 0..2)."""
    f32 = np.float32
    n = xs.shape[0]
    x2 = np.zeros((n, 32, 4, 4), f32)
    x2[:, 0:16] = xs
    x2[:, 16:32, :, 0:3] = xs[:, :, :, 1:4]
    x3 = np.zeros((n, 32, 4, 4), f32)
    x3[:, 0:16] = xs
    x3[:, 16:32, 0:3, :] = xs[:, :, 1:4, :]
    x2b = x2.astype(BF16)
    x3b = x3.astype(BF16)
    u2 = x2b.astype(f32)[:, :, :, 0:3].sum(axis=(2, 3)).T.astype(BF16)  # [32, n]
    u3 = x3b.astype(f32)[:, :, 0:3, :].sum(axis=(2, 3)).T.astype(BF16)
    x2 = x2b.transpose(1, 2, 3, 0).reshape(32, 16, n)
    x3 = x3b.transpose(1, 2, 3, 0).reshape(32, 16, n)
    return x2, x3, u2, u3


def _get_nc():
    if "nc" not in _cache:
        _cache["nc"] = _build()
    return _cache["nc"]


def kernel(**inputs) -> np.ndarray:
    from concourse.bass_utils import run_bass_kernel_spmd

    nc = _get_nc()
    x = np.asarray(inputs["x"], np.float32)
    w = _prep_weights(inputs)

    in_maps = []
    for c in range(NCORES):
        xs = x[c * BC : (c + 1) * BC]
        x2, x3, u2, u3 = _prep_x(xs)
        m = dict(w)
        m["x2"] = x2
        m["x3"] = x3
        m["u2"] = u2
        m["u3"] = u3
        in_maps.append(m)

    res = run_bass_kernel_spmd(nc, in_maps, core_ids=list(range(NCORES)))
    out = np.concatenate([r["out"] for r in res.results], axis=0)
    return out.astype(np.float32)
